# revision 38
# baseline (speedup 1.0000x reference)
"""DiscoBERT pooling + projection on 8 trn2 NeuronCores, transition parser on host.

Pipeline (per the reference):
  1. [device, data-parallel over 512 EDUs -> 64 per core]
     seq = sequence_output[:, :, :]            [N, 128, 768]
     a1  = tanh(seq @ W1 + b1)                 [N, 128, 100]
     a2  = a1 @ W2 + b2                        [N, 128]
     w   = softmax over s in 1..127 of (a2 * mask)   (CLS position 0 dropped)
     pooled = sum_s w[s] * seq[s]              [N, 768]
     enc = pooled @ Wp + bp                    [N, 256]
  2. [host] sequential 1023-step shift-reduce parser (inherently serial, tiny)
     -> returns stack[0]  [256]
"""

import numpy as np

import concourse.bass as bass
import concourse.bacc as bacc
import concourse.tile as tile
from concourse import mybir
from concourse.bass_utils import run_bass_kernel_spmd

# Problem constants (hardcoded per spec; kernel.py must be self-contained).
N_EDUS, SEQ, HB, H, A = 512, 128, 768, 256, 100
HALF = H // 2
N_CORES = 8
EPC = N_EDUS // N_CORES  # EDUs per core = 64
KH = HB // 128           # 6 h-chunks of 128
F32 = mybir.dt.float32

_CACHE = {}


def _build_nc(b2: float):
    """Build + compile the per-core Bass program (same program on all 8 cores)."""
    nc = bacc.Bacc("TRN2", target_bir_lowering=False, debug=False,
                   num_devices=N_CORES)

    d_seq = nc.dram_tensor("seq", [EPC, SEQ, HB], F32, kind="ExternalInput").ap()
    d_maskT = nc.dram_tensor("maskT", [SEQ, EPC], F32, kind="ExternalInput").ap()
    d_w1t = nc.dram_tensor("w1t", [128, KH * A], F32, kind="ExternalInput").ap()
    d_b1c = nc.dram_tensor("b1c", [A, 1], F32, kind="ExternalInput").ap()
    d_w2c = nc.dram_tensor("w2c", [A, 1], F32, kind="ExternalInput").ap()
    d_ident = nc.dram_tensor("ident", [128, 128], F32, kind="ExternalInput").ap()
    d_ones = nc.dram_tensor("ones", [128, 1], F32, kind="ExternalInput").ap()
    d_pool = nc.dram_tensor("pooled_out", [EPC, HB], F32,
                            kind="ExternalOutput").ap()
    d_sums = nc.dram_tensor("sums_out", [1, EPC], F32,
                            kind="ExternalOutput").ap()

    with tile.TileContext(nc) as tc:
        _emit(tc, d_seq, d_maskT, d_w1t, d_b1c, d_w2c,
              d_ident, d_ones, d_pool, d_sums, b2)

    nc.compile()
    return nc


def _emit(tc, d_seq, d_maskT, d_w1t, d_b1c, d_w2c,
          d_ident, d_ones, d_pool, d_sums, b2):
    nc = tc.nc
    from contextlib import ExitStack
    ctx = ExitStack()
    with ctx:
        consts = ctx.enter_context(tc.tile_pool(name="consts", bufs=1))
        seq_pool = ctx.enter_context(tc.tile_pool(name="seq", bufs=12))
        seqT_pool = ctx.enter_context(tc.tile_pool(name="seqT", bufs=2))
        a1s_pool = ctx.enter_context(tc.tile_pool(name="a1s", bufs=2))
        small_pool = ctx.enter_context(tc.tile_pool(name="small", bufs=1))
        stage_pool = ctx.enter_context(tc.tile_pool(name="stage", bufs=3))
        ps_tr = ctx.enter_context(tc.tile_pool(name="ps_tr", bufs=2, space="PSUM"))
        ps_a1 = ctx.enter_context(tc.tile_pool(name="ps_a1", bufs=2, space="PSUM"))
        ps_a2 = ctx.enter_context(tc.tile_pool(name="ps_a2", bufs=2, space="PSUM"))
        ps_row = ctx.enter_context(tc.tile_pool(name="ps_row", bufs=2, space="PSUM"))

        # ---- constants ----
        w1t = consts.tile([128, KH * A], F32)
        nc.sync.dma_start(w1t[:], d_w1t)
        b1c = consts.tile([A, 1], F32)
        nc.sync.dma_start(b1c[:], d_b1c)
        w2c = consts.tile([A, 1], F32)
        nc.sync.dma_start(w2c[:], d_w2c)
        ident = consts.tile([128, 128], F32)
        nc.sync.dma_start(ident[:], d_ident)
        ones_col = consts.tile([128, 1], F32)
        nc.sync.dma_start(ones_col[:], d_ones)
        maskT = consts.tile([SEQ, EPC], F32)
        nc.sync.dma_start(maskT[:], d_maskT)

        # ---- per-batch persistent tiles ----
        msk_sb = small_pool.tile([128, EPC], F32)
        expt = small_pool.tile([128, EPC], F32)

        # ---- main loop: 16 groups x 4 EDUs, software-pipelined so the PE
        # has front(g+1) transpose/a1 work while back(g)'s serial
        # a2->exp->pooled chain resolves ----
        NG = EPC // 4
        state = {}

        def front(g):
            seqT4 = seqT_pool.tile([128, KH * 4 * 128], F32, tag="seqT4")
            seqT4_r = seqT4[:].rearrange("p (k e s) -> p k e s", k=KH, e=4)
            seqs = []
            for e4 in range(4):
                e = 4 * g + e4
                seqt = seq_pool.tile([128, HB], F32, tag="seq")
                # alternate the two HWDGE rings so seq prefetch isn't
                # serialized behind one queue
                if e4 % 2 == 0:
                    nc.sync.dma_start(seqt[:], d_seq[e])
                else:
                    nc.scalar.dma_start(seqt[:], d_seq[e])
                seqs.append(seqt)
            # k-major: per h-chunk, transpose all 4 EDUs into one psum bank,
            # copy out contiguously, then immediately run that chunk's a1
            # matmul — interleaving real matmuls between transposes.
            a1p = ps_a1.tile([A, 4 * 128], F32, tag="a1p")
            for k in range(KH):
                ptr = ps_tr.tile([128, 4 * 128], F32, tag="ptr")
                for e4 in range(4):
                    nc.tensor.transpose(
                        ptr[:, 128 * e4:128 * (e4 + 1)],
                        seqs[e4][:, 128 * k:128 * (k + 1)],
                        ident[:],
                    )
                dst = seqT4_r[:, k, :, :]
                if k % 2 == 0:
                    nc.vector.tensor_copy(dst, ptr[:])
                else:
                    nc.scalar.copy(dst, ptr[:])
                nc.tensor.matmul(
                    a1p[:],
                    w1t[:, A * k:A * (k + 1)],
                    dst,
                    start=(k == 0),
                    stop=(k == KH - 1),
                )
            # tanh(. + b1) into [100, 512]
            a1s = a1s_pool.tile([A, 4 * 128], F32, tag="a1s")
            nc.scalar.activation(a1s[:], a1p[:],
                                 mybir.ActivationFunctionType.Tanh,
                                 bias=b1c[:])
            state[g] = (seqs, a1s)

        def back(g):
            seqs, a1s = state.pop(g)
            # a2 column per EDU: [128, 1] = a1s_e^T @ W2
            a2_ps = ps_a2.tile([128, 4], F32, tag="a2p")
            for e4 in range(4):
                nc.tensor.matmul(
                    a2_ps[:, e4:e4 + 1],
                    a1s[:, 128 * e4:128 * (e4 + 1)],
                    w2c[:],
                    start=True, stop=True,
                )
            # masked = (a2 + b2) * mask ; exp ; kill CLS row (s=0)
            sl = slice(4 * g, 4 * g + 4)
            nc.vector.scalar_tensor_tensor(
                msk_sb[:, sl], a2_ps[:], float(b2), maskT[:, sl],
                op0=mybir.AluOpType.add, op1=mybir.AluOpType.mult)
            nc.scalar.activation(expt[:, sl], msk_sb[:, sl],
                                 mybir.ActivationFunctionType.Exp)
            nc.vector.memset(expt[0:1, sl], 0.0)

            # unnormalized pooled row per EDU: [1, 768] = expt_e^T @ seq_e.
            # The 1-column weight load is ~free; moving operand is seq.
            # Both row pieces share one PSUM bank (partitions 0 and 32).
            for e4 in range(4):
                e = 4 * g + e4
                prow = ps_row.tile([33, 512], F32, tag="prow")
                nc.tensor.matmul(prow[0:1, 0:512], expt[:, e:e + 1],
                                 seqs[e4][:, 0:512], start=True, stop=True)
                nc.tensor.matmul(prow[32:33, 0:256], expt[:, e:e + 1],
                                 seqs[e4][:, 512:768], start=True, stop=True)
                stg = stage_pool.tile([1, HB], F32, tag="stg")
                nc.vector.tensor_copy(stg[:, 0:512], prow[0:1, 0:512])
                nc.scalar.copy(stg[:, 512:768], prow[32:33, 0:256])
                if e4 % 2 == 0:
                    nc.scalar.dma_start(d_pool[e:e + 1, :], stg[:])
                else:
                    nc.sync.dma_start(d_pool[e:e + 1, :], stg[:])

        front(0)
        for g in range(1, NG):
            front(g)
            back(g - 1)
        back(NG - 1)

        # ---- batch tail: per-EDU softmax denominators ----
        sums_ps = ps_row.tile([33, 512], F32, tag="prow")
        nc.tensor.matmul(sums_ps[0:1, 0:EPC], ones_col[:], expt[:],
                         start=True, stop=True)
        sums_sb = small_pool.tile([1, EPC], F32)
        nc.vector.tensor_copy(sums_sb[:], sums_ps[0:1, 0:EPC])
        nc.scalar.dma_start(d_sums, sums_sb[:])


def _prep_inputs(sequence_output, attention_mask, attn1_w, attn1_b, attn2_w,
                 attn2_b, project_w, project_b):
    """Host-side sharding + weight relayout. Returns in_maps for 8 cores."""
    seq = np.ascontiguousarray(np.asarray(sequence_output, np.float32))
    mask_f = np.asarray(attention_mask).astype(np.float32)        # [512, 128]

    w1t = np.ascontiguousarray(
        np.asarray(attn1_w, np.float32).reshape(KH, 128, A)
        .transpose(1, 0, 2).reshape(128, KH * A))
    b1c = np.ascontiguousarray(np.asarray(attn1_b, np.float32).reshape(A, 1))
    w2c = np.ascontiguousarray(np.asarray(attn2_w, np.float32).reshape(A, 1))
    ident = np.eye(128, dtype=np.float32)
    ones = np.ones((128, 1), dtype=np.float32)

    in_maps = []
    for c in range(N_CORES):
        sl = slice(c * EPC, (c + 1) * EPC)
        in_maps.append({
            "seq": seq[sl],
            "maskT": np.ascontiguousarray(mask_f[sl].T),
            "w1t": w1t, "b1c": b1c, "w2c": w2c,
            "ident": ident, "ones": ones,
        })
    return in_maps


def run_device_encode(sequence_output, attention_mask, attn1_w, attn1_b,
                      attn2_w, attn2_b, project_w, project_b, trace=False):
    """Run the 8-core bass kernel; returns (enc [512, 256], results obj)."""
    b2 = float(np.asarray(attn2_b).reshape(-1)[0])
    if _CACHE.get("b2") != b2:
        _CACHE["nc"] = _build_nc(b2)
        _CACHE["b2"] = b2
    nc = _CACHE["nc"]
    in_maps = _prep_inputs(sequence_output, attention_mask, attn1_w, attn1_b,
                           attn2_w, attn2_b, project_w, project_b)
    res = run_bass_kernel_spmd(nc, in_maps, list(range(N_CORES)), trace=trace)
    pooled = np.concatenate(
        [res.results[c]["pooled_out"] for c in range(N_CORES)], axis=0)
    sums = np.concatenate(
        [res.results[c]["sums_out"][0] for c in range(N_CORES)], axis=0)
    # tiny host epilogue: normalize + project (0.2 GFLOP)
    pooled = pooled / sums[:, None]
    enc = pooled @ np.asarray(project_w, np.float32) \
        + np.asarray(project_b, np.float32)
    return enc.astype(np.float32), res


def _host_parser(enc, missing_node, action_w, action_b, tree_w, tree_b):
    """Sequential shift-reduce parser, mirrors the reference's scan exactly
    (run on host CPU via jax; inherently serial, ~0.1% of total FLOPs)."""
    import jax
    import jax.numpy as jnp

    cpu = jax.local_devices(backend="cpu")[0]
    with jax.default_device(cpu):
        enc_j = jnp.asarray(enc, jnp.float32)
        missing = jnp.asarray(missing_node, jnp.float32)
        aw = jnp.asarray(action_w, jnp.float32)
        ab = jnp.asarray(action_b, jnp.float32)
        tw = jnp.asarray(tree_w, jnp.float32)
        tb = jnp.asarray(tree_b, jnp.float32)
        N = enc_j.shape[0]
        NEG = jnp.asarray(-1e9, jnp.float32)

        def treelstm(e1, e2):
            h1, c1 = e1[:HALF], e1[HALF:]
            h2, c2 = e2[:HALF], e2[HALF:]
            gg = jnp.concatenate([h1, h2]) @ tw + tb
            i, f1, f2, o, u = jnp.split(gg, 5)
            c = (jax.nn.sigmoid(i) * jnp.tanh(u) + jax.nn.sigmoid(f1) * c1
                 + jax.nn.sigmoid(f2) * c2)
            h = jax.nn.sigmoid(o) * jnp.tanh(c)
            return jnp.concatenate([h, c])

        def step(carry, _):
            stack, sp, bi = carry
            s1 = jnp.where(sp >= 2, stack[jnp.maximum(sp - 2, 0)], missing)
            s0 = jnp.where(sp >= 1, stack[jnp.maximum(sp - 1, 0)], missing)
            b = jnp.where(bi < N, enc_j[jnp.minimum(bi, N - 1)], missing)
            feat = jnp.concatenate([s1, s0, b])
            scores = feat @ aw + ab
            legal = jnp.stack([bi < N, sp >= 2])
            shift = jnp.argmax(jnp.where(legal, scores, NEG)) == 0
            merged = treelstm(s1, s0)
            new_idx = jnp.where(shift, sp, jnp.maximum(sp - 2, 0))
            new_val = jnp.where(shift, b, merged)
            stack = stack.at[new_idx].set(new_val)
            sp = jnp.where(shift, sp + 1, sp - 1)
            bi = jnp.where(shift, bi + 1, bi)
            return (stack, sp, bi), None

        stack0 = jnp.zeros((N, H), jnp.float32)
        (stack, sp, bi), _ = jax.lax.scan(
            step, (stack0, jnp.int32(0), jnp.int32(0)), None, length=2 * N - 1)
        return np.asarray(stack[0])


def kernel(**inputs):
    enc, _ = run_device_encode(
        inputs["sequence_output"], inputs["attention_mask"],
        inputs["attn1_w"], inputs["attn1_b"], inputs["attn2_w"],
        inputs["attn2_b"], inputs["project_w"], inputs["project_b"])
    return _host_parser(enc, inputs["missing_node"], inputs["action_w"],
                        inputs["action_b"], inputs["tree_w"], inputs["tree_b"])


# revision 41
# speedup vs baseline: 1.2244x; 1.2244x over previous
"""DiscoBERT pooling + projection on 8 trn2 NeuronCores, transition parser on host.

Pipeline (per the reference):
  1. [device, data-parallel over 512 EDUs -> 64 per core]
     seq = sequence_output[:, :, :]            [N, 128, 768]
     a1  = tanh(seq @ W1 + b1)                 [N, 128, 100]
     a2  = a1 @ W2 + b2                        [N, 128]
     w   = softmax over s in 1..127 of (a2 * mask)   (CLS position 0 dropped)
     pooled = sum_s w[s] * seq[s]              [N, 768]
     enc = pooled @ Wp + bp                    [N, 256]
  2. [host] sequential 1023-step shift-reduce parser (inherently serial, tiny)
     -> returns stack[0]  [256]
"""

import numpy as np

import concourse.bass as bass
import concourse.bacc as bacc
import concourse.tile as tile
from concourse import mybir
from concourse.bass_utils import run_bass_kernel_spmd

# Problem constants (hardcoded per spec; kernel.py must be self-contained).
N_EDUS, SEQ, HB, H, A = 512, 128, 768, 256, 100
HALF = H // 2
N_CORES = 8
EPC = N_EDUS // N_CORES  # EDUs per core = 64
KH = HB // 128           # 6 h-chunks of 128
F32 = mybir.dt.float32

_CACHE = {}


def _build_nc(b2: float):
    """Build + compile the per-core Bass program (same program on all 8 cores)."""
    nc = bacc.Bacc("TRN2", target_bir_lowering=False, debug=False,
                   num_devices=N_CORES)

    d_seq = nc.dram_tensor("seq", [EPC, SEQ, HB], F32, kind="ExternalInput").ap()
    d_maskT = nc.dram_tensor("maskT", [SEQ, EPC], F32, kind="ExternalInput").ap()
    d_w1t = nc.dram_tensor("w1t", [128, KH * A], F32, kind="ExternalInput").ap()
    d_b1c = nc.dram_tensor("b1c", [A, 1], F32, kind="ExternalInput").ap()
    d_w2c = nc.dram_tensor("w2c", [A, 1], F32, kind="ExternalInput").ap()
    d_ident = nc.dram_tensor("ident", [128, 128], F32, kind="ExternalInput").ap()
    d_ones = nc.dram_tensor("ones", [128, 1], F32, kind="ExternalInput").ap()
    d_pool = nc.dram_tensor("pooled_out", [EPC, HB], F32,
                            kind="ExternalOutput").ap()
    d_sums = nc.dram_tensor("sums_out", [1, EPC], F32,
                            kind="ExternalOutput").ap()

    with tile.TileContext(nc) as tc:
        _emit(tc, d_seq, d_maskT, d_w1t, d_b1c, d_w2c,
              d_ident, d_ones, d_pool, d_sums, b2)

    nc.compile()
    return nc


def _emit(tc, d_seq, d_maskT, d_w1t, d_b1c, d_w2c,
          d_ident, d_ones, d_pool, d_sums, b2):
    nc = tc.nc
    from contextlib import ExitStack
    ctx = ExitStack()
    with ctx:
        consts = ctx.enter_context(tc.tile_pool(name="consts", bufs=1))
        seq_pool = ctx.enter_context(tc.tile_pool(name="seq", bufs=12))
        seqT_pool = ctx.enter_context(tc.tile_pool(name="seqT", bufs=2))
        a1s_pool = ctx.enter_context(tc.tile_pool(name="a1s", bufs=2))
        small_pool = ctx.enter_context(tc.tile_pool(name="small", bufs=1))
        stage_pool = ctx.enter_context(tc.tile_pool(name="stage", bufs=3))
        ps_tr = ctx.enter_context(tc.tile_pool(name="ps_tr", bufs=2, space="PSUM"))
        ps_a1 = ctx.enter_context(tc.tile_pool(name="ps_a1", bufs=2, space="PSUM"))
        ps_a2 = ctx.enter_context(tc.tile_pool(name="ps_a2", bufs=2, space="PSUM"))
        ps_row = ctx.enter_context(tc.tile_pool(name="ps_row", bufs=2, space="PSUM"))

        # ---- group 0's seq loads first: they gate the first transposes,
        # and the sync HWDGE ring drains FIFO per issuing engine ----
        seqs0 = []
        for e4 in range(4):
            seqt = seq_pool.tile([128, HB], F32, tag="seq")
            nc.sync.dma_start(seqt[:], d_seq[e4])
            seqs0.append(seqt)

        # ---- constants ----
        w1t = consts.tile([128, KH * A], F32)
        nc.sync.dma_start(w1t[:], d_w1t)
        b1c = consts.tile([A, 1], F32)
        nc.sync.dma_start(b1c[:], d_b1c)
        w2c = consts.tile([A, 1], F32)
        nc.sync.dma_start(w2c[:], d_w2c)
        ident = consts.tile([128, 128], F32)
        nc.sync.dma_start(ident[:], d_ident)
        ones_col = consts.tile([128, 1], F32)
        nc.sync.dma_start(ones_col[:], d_ones)
        maskT = consts.tile([SEQ, EPC], F32)
        nc.sync.dma_start(maskT[:], d_maskT)

        # ---- per-batch persistent tiles ----
        msk_sb = small_pool.tile([128, EPC], F32)
        expt = small_pool.tile([128, EPC], F32)

        # ---- main loop: 16 groups x 4 EDUs, software-pipelined so the PE
        # has front(g+1) transpose/a1 work while back(g)'s serial
        # a2->exp->pooled chain resolves ----
        NG = EPC // 4
        state = {}

        def front(g):
            seqT4 = seqT_pool.tile([128, KH * 4 * 128], F32, tag="seqT4")
            seqT4_r = seqT4[:].rearrange("p (k e s) -> p k e s", k=KH, e=4)
            if g == 0:
                seqs = seqs0
            else:
                seqs = []
                for e4 in range(4):
                    e = 4 * g + e4
                    seqt = seq_pool.tile([128, HB], F32, tag="seq")
                    nc.sync.dma_start(seqt[:], d_seq[e])
                    seqs.append(seqt)
            # k-major: per h-chunk, transpose all 4 EDUs into one psum bank,
            # copy out contiguously, then immediately run that chunk's a1
            # matmul — interleaving real matmuls between transposes.
            a1p = ps_a1.tile([A, 4 * 128], F32, tag="a1p")
            for k in range(KH):
                ptr = ps_tr.tile([128, 4 * 128], F32, tag="ptr")
                for e4 in range(4):
                    nc.tensor.transpose(
                        ptr[:, 128 * e4:128 * (e4 + 1)],
                        seqs[e4][:, 128 * k:128 * (k + 1)],
                        ident[:],
                    )
                dst = seqT4_r[:, k, :, :]
                if k % 2 == 0:
                    nc.vector.tensor_copy(dst, ptr[:])
                else:
                    nc.scalar.copy(dst, ptr[:])
                nc.tensor.matmul(
                    a1p[:],
                    w1t[:, A * k:A * (k + 1)],
                    dst,
                    start=(k == 0),
                    stop=(k == KH - 1),
                )
            # tanh(. + b1) into [100, 512]
            a1s = a1s_pool.tile([A, 4 * 128], F32, tag="a1s")
            nc.scalar.activation(a1s[:], a1p[:],
                                 mybir.ActivationFunctionType.Tanh,
                                 bias=b1c[:])
            state[g] = (seqs, a1s)

        def back(g):
            seqs, a1s = state.pop(g)
            # a2 column per EDU: [128, 1] = a1s_e^T @ W2
            a2_ps = ps_a2.tile([128, 4], F32, tag="a2p")
            for e4 in range(4):
                nc.tensor.matmul(
                    a2_ps[:, e4:e4 + 1],
                    a1s[:, 128 * e4:128 * (e4 + 1)],
                    w2c[:],
                    start=True, stop=True,
                )
            # masked = (a2 + b2) * mask ; exp ; kill CLS row (s=0)
            sl = slice(4 * g, 4 * g + 4)
            nc.vector.scalar_tensor_tensor(
                msk_sb[:, sl], a2_ps[:], float(b2), maskT[:, sl],
                op0=mybir.AluOpType.add, op1=mybir.AluOpType.mult)
            nc.scalar.activation(expt[:, sl], msk_sb[:, sl],
                                 mybir.ActivationFunctionType.Exp)
            nc.vector.memset(expt[0:1, sl], 0.0)

            # unnormalized pooled row per EDU: [1, 768] = expt_e^T @ seq_e.
            # The 1-column weight load is ~free; moving operand is seq.
            # Both row pieces share one PSUM bank (partitions 0 and 32).
            for e4 in range(4):
                e = 4 * g + e4
                prow = ps_row.tile([33, 512], F32, tag="prow")
                nc.tensor.matmul(prow[0:1, 0:512], expt[:, e:e + 1],
                                 seqs[e4][:, 0:512], start=True, stop=True)
                nc.tensor.matmul(prow[32:33, 0:256], expt[:, e:e + 1],
                                 seqs[e4][:, 512:768], start=True, stop=True)
                stg = stage_pool.tile([1, HB], F32, tag="stg")
                nc.vector.tensor_copy(stg[:, 0:512], prow[0:1, 0:512])
                nc.scalar.copy(stg[:, 512:768], prow[32:33, 0:256])
                if e4 % 2 == 0:
                    nc.scalar.dma_start(d_pool[e:e + 1, :], stg[:])
                else:
                    nc.sync.dma_start(d_pool[e:e + 1, :], stg[:])

        front(0)
        for g in range(1, NG):
            front(g)
            back(g - 1)
        back(NG - 1)

        # ---- batch tail: per-EDU softmax denominators ----
        sums_ps = ps_row.tile([33, 512], F32, tag="prow")
        nc.tensor.matmul(sums_ps[0:1, 0:EPC], ones_col[:], expt[:],
                         start=True, stop=True)
        sums_sb = small_pool.tile([1, EPC], F32)
        nc.vector.tensor_copy(sums_sb[:], sums_ps[0:1, 0:EPC])
        nc.scalar.dma_start(d_sums, sums_sb[:])


def _prep_inputs(sequence_output, attention_mask, attn1_w, attn1_b, attn2_w,
                 attn2_b, project_w, project_b):
    """Host-side sharding + weight relayout. Returns in_maps for 8 cores."""
    seq = np.ascontiguousarray(np.asarray(sequence_output, np.float32))
    mask_f = np.asarray(attention_mask).astype(np.float32)        # [512, 128]

    w1t = np.ascontiguousarray(
        np.asarray(attn1_w, np.float32).reshape(KH, 128, A)
        .transpose(1, 0, 2).reshape(128, KH * A))
    b1c = np.ascontiguousarray(np.asarray(attn1_b, np.float32).reshape(A, 1))
    w2c = np.ascontiguousarray(np.asarray(attn2_w, np.float32).reshape(A, 1))
    ident = np.eye(128, dtype=np.float32)
    ones = np.ones((128, 1), dtype=np.float32)

    in_maps = []
    for c in range(N_CORES):
        sl = slice(c * EPC, (c + 1) * EPC)
        in_maps.append({
            "seq": seq[sl],
            "maskT": np.ascontiguousarray(mask_f[sl].T),
            "w1t": w1t, "b1c": b1c, "w2c": w2c,
            "ident": ident, "ones": ones,
        })
    return in_maps


def run_device_encode(sequence_output, attention_mask, attn1_w, attn1_b,
                      attn2_w, attn2_b, project_w, project_b, trace=False):
    """Run the 8-core bass kernel; returns (enc [512, 256], results obj)."""
    b2 = float(np.asarray(attn2_b).reshape(-1)[0])
    if _CACHE.get("b2") != b2:
        _CACHE["nc"] = _build_nc(b2)
        _CACHE["b2"] = b2
    nc = _CACHE["nc"]
    in_maps = _prep_inputs(sequence_output, attention_mask, attn1_w, attn1_b,
                           attn2_w, attn2_b, project_w, project_b)
    res = run_bass_kernel_spmd(nc, in_maps, list(range(N_CORES)), trace=trace)
    pooled = np.concatenate(
        [res.results[c]["pooled_out"] for c in range(N_CORES)], axis=0)
    sums = np.concatenate(
        [res.results[c]["sums_out"][0] for c in range(N_CORES)], axis=0)
    # tiny host epilogue: normalize + project (0.2 GFLOP)
    pooled = pooled / sums[:, None]
    enc = pooled @ np.asarray(project_w, np.float32) \
        + np.asarray(project_b, np.float32)
    return enc.astype(np.float32), res


def _host_parser(enc, missing_node, action_w, action_b, tree_w, tree_b):
    """Sequential shift-reduce parser, mirrors the reference's scan exactly
    (run on host CPU via jax; inherently serial, ~0.1% of total FLOPs)."""
    import jax
    import jax.numpy as jnp

    cpu = jax.local_devices(backend="cpu")[0]
    with jax.default_device(cpu):
        enc_j = jnp.asarray(enc, jnp.float32)
        missing = jnp.asarray(missing_node, jnp.float32)
        aw = jnp.asarray(action_w, jnp.float32)
        ab = jnp.asarray(action_b, jnp.float32)
        tw = jnp.asarray(tree_w, jnp.float32)
        tb = jnp.asarray(tree_b, jnp.float32)
        N = enc_j.shape[0]
        NEG = jnp.asarray(-1e9, jnp.float32)

        def treelstm(e1, e2):
            h1, c1 = e1[:HALF], e1[HALF:]
            h2, c2 = e2[:HALF], e2[HALF:]
            gg = jnp.concatenate([h1, h2]) @ tw + tb
            i, f1, f2, o, u = jnp.split(gg, 5)
            c = (jax.nn.sigmoid(i) * jnp.tanh(u) + jax.nn.sigmoid(f1) * c1
                 + jax.nn.sigmoid(f2) * c2)
            h = jax.nn.sigmoid(o) * jnp.tanh(c)
            return jnp.concatenate([h, c])

        def step(carry, _):
            stack, sp, bi = carry
            s1 = jnp.where(sp >= 2, stack[jnp.maximum(sp - 2, 0)], missing)
            s0 = jnp.where(sp >= 1, stack[jnp.maximum(sp - 1, 0)], missing)
            b = jnp.where(bi < N, enc_j[jnp.minimum(bi, N - 1)], missing)
            feat = jnp.concatenate([s1, s0, b])
            scores = feat @ aw + ab
            legal = jnp.stack([bi < N, sp >= 2])
            shift = jnp.argmax(jnp.where(legal, scores, NEG)) == 0
            merged = treelstm(s1, s0)
            new_idx = jnp.where(shift, sp, jnp.maximum(sp - 2, 0))
            new_val = jnp.where(shift, b, merged)
            stack = stack.at[new_idx].set(new_val)
            sp = jnp.where(shift, sp + 1, sp - 1)
            bi = jnp.where(shift, bi + 1, bi)
            return (stack, sp, bi), None

        stack0 = jnp.zeros((N, H), jnp.float32)
        (stack, sp, bi), _ = jax.lax.scan(
            step, (stack0, jnp.int32(0), jnp.int32(0)), None, length=2 * N - 1)
        return np.asarray(stack[0])


def kernel(**inputs):
    enc, _ = run_device_encode(
        inputs["sequence_output"], inputs["attention_mask"],
        inputs["attn1_w"], inputs["attn1_b"], inputs["attn2_w"],
        inputs["attn2_b"], inputs["project_w"], inputs["project_b"])
    return _host_parser(enc, inputs["missing_node"], inputs["action_w"],
                        inputs["action_b"], inputs["tree_w"], inputs["tree_b"])
